# revision 1
# baseline (speedup 1.0000x reference)
"""Trainium2 Bass kernel for nn_Bdfdv_51170240364850 (gnn_message_passing).

Computes, for mode pairs (il, im) with im <= il (L1 = 5 modes each way) and
spatial/velocity grid (nx=1024, nv=512):

  D[il,im] = base + (-1j)*im*bx*F[il,im] + cB*bm*F[il,im+1]
             + [im==0] Re(cC*bp*F[il,1])
  base     = 0.5*bm*F[il,im-1]  (il>=1, 1<=im<=il)   else  D0[il,im]

with bx = b[:,0], bm = b[:,1]+1j b[:,2], bp = conj(bm),
cB = -(il-im)(il+im+1)/2, cC = -il(il+1).

Strategy: pure data-parallel over nx across 8 NeuronCores (nx=128 per core,
mapped onto the 128 SBUF partitions). All per-x scalar coefficient products
(constants x b-columns) are precomputed host-side into a small [128, 48]
table, so every device-side term is a single fused
scalar_tensor_tensor(out = in0 * scal + in1) instruction. Mode (il,0) folds
the term4 contribution into the cB coefficients (cC = 2*cB(il,0)).

Inputs are packed host-side into one [128, CIN] f32 array per core (valid
mode slices only: (0,0) passes through host-side); output is one
[128, COUT] f32 array per core, unpacked host-side into the complex64
(5,5,1024,512) result.
"""

import numpy as np

import bass_rust
import concourse.bass as bass
import concourse.tile as tile
from concourse import mybir
from concourse.bass_utils import run_bass_kernel_spmd
from concourse.vector_clock import ScopedClock

L1 = 5
NX = 1024
NV = 512
NCORES = 8
XS = NX // NCORES  # 128, = SBUF partitions

F32 = mybir.dt.float32

# ---------------------------------------------------------------------------
# scal table columns
H1, H2, NH2 = 0, 1, 2          # 0.5*b1, 0.5*b2, -0.5*b2


def A_P(m):                    # +m*b0  (m = 1..4)
    return 3 + (m - 1)


def A_N(m):                    # -m*b0
    return 7 + (m - 1)


def Q1(il):                    # 3*cB0*b1, cB0 = -il(il+1)/2
    return 11 + (il - 1)


def Q2(il):                    # cB0*b2
    return 15 + (il - 1)


def R1(il):                    # cB0*b1
    return 19 + (il - 1)


CB_PAIRS = [(2, 1), (3, 1), (3, 2), (4, 1), (4, 2), (4, 3)]


def CB1(il, im):               # cB*b1
    return 23 + 3 * CB_PAIRS.index((il, im))


def CB2(il, im):               # cB*b2
    return 24 + 3 * CB_PAIRS.index((il, im))


def NCB2(il, im):              # -cB*b2
    return 25 + 3 * CB_PAIRS.index((il, im))


NSCAL = 48  # 41 used, padded

# packed input layout: [scal (NSCAL) | row blocks il=1..4]
# row block: Fr slots (il+1), Fi slots (il+1), D0r, D0i  -- each slot NV cols
IN_OFF = {}
_o = NSCAL
for _il in range(1, L1):
    IN_OFF[_il] = _o
    _o += (2 * (_il + 1) + 2) * NV
CIN = _o

# packed output layout: row blocks il=1..4, each: Dr slots (il+1), Di slots
OUT_OFF = {}
_o = 0
for _il in range(1, L1):
    OUT_OFF[_il] = _o
    _o += 2 * (_il + 1) * NV
COUT = _o


def _cB(il, im):
    return -(il - im) * (il + im + 1) / 2.0


def build_scal(b_sh):
    """b_sh: [XS, 3] float32 -> [XS, NSCAL] float32 coefficient table."""
    b0, b1, b2 = b_sh[:, 0], b_sh[:, 1], b_sh[:, 2]
    s = np.zeros((XS, NSCAL), np.float32)
    s[:, H1] = 0.5 * b1
    s[:, H2] = 0.5 * b2
    s[:, NH2] = -0.5 * b2
    for m in range(1, L1):
        s[:, A_P(m)] = m * b0
        s[:, A_N(m)] = -m * b0
    for il in range(1, L1):
        cB0 = _cB(il, 0)
        s[:, Q1(il)] = 3.0 * cB0 * b1
        s[:, Q2(il)] = cB0 * b2
        s[:, R1(il)] = cB0 * b1
    for (il, im) in CB_PAIRS:
        cB = _cB(il, im)
        s[:, CB1(il, im)] = cB * b1
        s[:, CB2(il, im)] = cB * b2
        s[:, NCB2(il, im)] = -cB * b2
    return s


# ---------------------------------------------------------------------------
# The walrus build in this container rejects instructions carrying more than
# ONE sync-wait ("Too many sync wait commands", setupSyncWait in
# CoreV2/V3GenImpl). Tile's scheduler routinely attaches several. Post-pass:
# hoist all but the last wait of each instruction onto same-engine NOPs
# inserted immediately before it (same basic block, so per-engine program
# order is preserved).
def split_multiwaits(nc):
    for f in nc.m.functions:
        for blk in f.blocks:
            new = []
            changed = False
            for ins in blk.instructions:
                si = ins.sync_info
                if si is not None and len(si.on_wait) > 1:
                    waits = list(si.on_wait)
                    for w in waits[:-1]:
                        nop = mybir.InstNoOp(
                            name=nc.get_next_instruction_name(),
                            engine=ins.engine,
                            bass_nofuse=True,
                            sync_info=mybir.SyncInfo(on_wait=[w],
                                                     on_update=[]),
                        )
                        new.append(nop)
                    ins.sync_info = bass_rust.SyncInfo(
                        on_wait=[waits[-1]], on_update=list(si.on_update))
                    changed = True
                new.append(ins)
            if changed:
                blk.instructions = new


# ---------------------------------------------------------------------------
def _pair(ap, step_elems, nblocks=2):
    """Turn a contiguous [P, L] AP into [P, nblocks, L] with the given
    element step between blocks (may be negative)."""
    c = ap.copy()
    v = c.ap
    last = v.pop()
    v.append((step_elems, nblocks))
    v.append(tuple(last))
    c.ap = v
    return c


def build_bass(split=True):
    """Pair-merged elementwise kernel.

    Per il-row SBUF layout: in-tile [nfi | fr | fi] (ns slots each,
    ns = il+1), out-tile [dr | di]. nfi = -fi (one ACT negate per row) lets
    every b2-coefficient op run as ONE fused scalar_tensor_tensor covering
    BOTH the Dr and Di halves (2-block strided APs, same per-x scalar
    column), halving DVE instruction count. ACT produces the set-term heads
    (b1 pair) and the negates; DVE runs all fused accumulates. Input DMAs
    are chained so row 1 lands first and compute ramps early.
    """
    from bass_rust import add_dep_helper

    MULT = mybir.AluOpType.mult
    ADD = mybir.AluOpType.add

    nc = bass.Bass()
    pin = nc.dram_tensor("pin", [XS, CIN], F32, kind="ExternalInput").ap()
    pout = nc.dram_tensor("pout", [XS, COUT], F32, kind="ExternalOutput").ap()

    with tile.TileContext(nc) as tc:
        with tc.tile_pool(name="m", bufs=1) as pool:
            scal = pool.tile([XS, NSCAL], F32, tag="scal")
            prev_dma = nc.sync.dma_start(scal[:], pin[:, 0:NSCAL])

            def sc(col):
                return scal[:, col:col + 1]

            def chain(d):
                nonlocal prev_dma
                add_dep_helper(d.ins, prev_dma.ins,
                               reason="serialize input DMAs")
                prev_dma = d

            row_in = {}
            row_d0 = {}
            for il in range(1, L1):
                ns = il + 1
                t = pool.tile([XS, 3 * ns * NV], F32, tag=f"in{il}")
                d0 = pool.tile([XS, 2 * NV], F32, tag=f"d0_{il}")
                # pin row block: fr slots, fi slots, d0r, d0i
                o = IN_OFF[il]
                chain(nc.sync.dma_start(t[:, ns * NV:3 * ns * NV],
                                        pin[:, o:o + 2 * ns * NV]))
                chain(nc.sync.dma_start(
                    d0[:], pin[:, o + 2 * ns * NV:o + (2 * ns + 2) * NV]))
                row_in[il] = t
                row_d0[il] = d0

            for il in range(1, L1):
                t = row_in[il]
                d0 = row_d0[il]
                ns = il + 1
                nfi = t[:, 0:ns * NV]
                fr = t[:, ns * NV:2 * ns * NV]
                fi = t[:, 2 * ns * NV:3 * ns * NV]
                d0r = d0[:, 0:NV]
                d0i = d0[:, NV:2 * NV]

                to = pool.tile([XS, 2 * ns * NV], F32, tag=f"out{il}")
                dr = to[:, 0:ns * NV]
                di = to[:, ns * NV:2 * ns * NV]

                S = ns * NV  # slot-block stride (elements)

                def sl(buf, k, n=1):
                    return buf[:, k * NV:(k + n) * NV]

                # ACT: nfi = -fi
                nc.scalar.mul(nfi, fi, -1.0)
                # ACT set-b1 head pair:
                #   dr[1..il] = 0.5*b1*fr[0..il-1]; di[1..il] = 0.5*b1*fi[..]
                nc.scalar.mul(_pair(sl(dr, 1, il), S), _pair(sl(fr, 0, il), S),
                              sc(H1))
                # DVE set-b2 pair: dr += 0.5*b2*nfi[0..il-1];
                #                  di += 0.5*b2*fr[0..il-1]
                nc.vector.scalar_tensor_tensor(
                    _pair(sl(dr, 1, il), S), _pair(sl(nfi, 0, il), S),
                    sc(H2), _pair(sl(dr, 1, il), S), MULT, ADD)
                # im=0 bases (distinct b1 scalars -> two singles)
                nc.vector.scalar_tensor_tensor(
                    sl(dr, 0), sl(fr, 1), sc(Q1(il)), d0r, MULT, ADD)
                nc.vector.scalar_tensor_tensor(
                    sl(di, 0), sl(fi, 1), sc(R1(il)), d0i, MULT, ADD)
                # im=0 b2 terms (no negative-stride pairing on HW)
                nc.vector.scalar_tensor_tensor(
                    sl(dr, 0), sl(fi, 1), sc(Q2(il)), sl(dr, 0), MULT, ADD)
                nc.vector.scalar_tensor_tensor(
                    sl(di, 0), sl(fr, 1), sc(Q2(il)), sl(di, 0), MULT, ADD)
                # cB pairs (im=1..il-1)
                for im in range(1, il):
                    nc.vector.scalar_tensor_tensor(
                        _pair(sl(dr, im), S), _pair(sl(fr, im + 1), S),
                        sc(CB1(il, im)), _pair(sl(dr, im), S), MULT, ADD)
                    nc.vector.scalar_tensor_tensor(
                        _pair(sl(dr, im), S), _pair(sl(nfi, im + 1), S),
                        sc(CB2(il, im)), _pair(sl(dr, im), S), MULT, ADD)
                # cA singles
                for im in range(1, il + 1):
                    nc.vector.scalar_tensor_tensor(
                        sl(dr, im), sl(fi, im), sc(A_P(im)), sl(dr, im),
                        MULT, ADD)
                    nc.vector.scalar_tensor_tensor(
                        sl(di, im), sl(fr, im), sc(A_N(im)), sl(di, im),
                        MULT, ADD)

                nc.sync.dma_start(
                    pout[:, OUT_OFF[il]:OUT_OFF[il] + ns * NV], dr)
                nc.sync.dma_start(
                    pout[:, OUT_OFF[il] + ns * NV:OUT_OFF[il] + 2 * ns * NV],
                    di)

    if split:
        split_multiwaits(nc)
    return nc


# ---------------------------------------------------------------------------
def pack_inputs(prev_f_re, prev_f_im, delta0_re, delta0_im, b):
    """-> list of per-core {'pin': [XS, CIN] f32}."""
    in_maps = []
    for c in range(NCORES):
        X = slice(c * XS, (c + 1) * XS)
        p = np.empty((XS, CIN), np.float32)
        p[:, :NSCAL] = 0.0
        p[:, :NSCAL][:, :41] = build_scal(np.asarray(b[X], np.float32))[:, :41]
        for il in range(1, L1):
            o = IN_OFF[il]
            ns = il + 1
            p[:, o:o + ns * NV] = (
                np.asarray(prev_f_re[il, :ns, X, :], np.float32)
                .transpose(1, 0, 2).reshape(XS, ns * NV))
            o += ns * NV
            p[:, o:o + ns * NV] = (
                np.asarray(prev_f_im[il, :ns, X, :], np.float32)
                .transpose(1, 0, 2).reshape(XS, ns * NV))
            o += ns * NV
            p[:, o:o + NV] = np.asarray(delta0_re[il, 0, X, :], np.float32)
            o += NV
            p[:, o:o + NV] = np.asarray(delta0_im[il, 0, X, :], np.float32)
        in_maps.append({"pin": p})
    return in_maps


def unpack_outputs(results, delta0_re, delta0_im):
    out = np.zeros((L1, L1, NX, NV), np.complex64)
    out[0, 0] = np.asarray(delta0_re[0, 0]) + 1j * np.asarray(delta0_im[0, 0])
    for c in range(NCORES):
        X = slice(c * XS, (c + 1) * XS)
        p = results[c]["pout"]
        for il in range(1, L1):
            o = OUT_OFF[il]
            ns = il + 1
            dr = p[:, o:o + ns * NV].reshape(XS, ns, NV).transpose(1, 0, 2)
            di = (p[:, o + ns * NV:o + 2 * ns * NV]
                  .reshape(XS, ns, NV).transpose(1, 0, 2))
            out[il, :ns, X, :] = dr + 1j * di
    return out


_NC_CACHE = None


def get_nc():
    global _NC_CACHE
    if _NC_CACHE is None:
        _NC_CACHE = build_bass()
    return _NC_CACHE


def kernel(prev_f_re, prev_f_im, delta0_re, delta0_im, b, v):
    in_maps = pack_inputs(prev_f_re, prev_f_im, delta0_re, delta0_im, b)
    res = run_bass_kernel_spmd(get_nc(), in_maps, list(range(NCORES)))
    return unpack_outputs(res.results, delta0_re, delta0_im)



# revision 6
# speedup vs baseline: 1.2865x; 1.2865x over previous
"""Trainium2 Bass kernel for nn_Bdfdv_51170240364850 (gnn_message_passing).

Computes, for mode pairs (il, im) with im <= il (L1 = 5 modes each way) and
spatial/velocity grid (nx=1024, nv=512):

  D[il,im] = base + (-1j)*im*bx*F[il,im] + cB*bm*F[il,im+1]
             + [im==0] Re(cC*bp*F[il,1])
  base     = 0.5*bm*F[il,im-1]  (il>=1, 1<=im<=il)   else  D0[il,im]

with bx = b[:,0], bm = b[:,1]+1j b[:,2], bp = conj(bm),
cB = -(il-im)(il+im+1)/2, cC = -il(il+1).

Strategy: pure data-parallel over nx across 8 NeuronCores (nx=128 per core,
mapped onto the 128 SBUF partitions). All per-x scalar coefficient products
(constants x b-columns) are precomputed host-side into a small [128, 48]
table, so every device-side term is a single fused
scalar_tensor_tensor(out = in0 * scal + in1) instruction. Mode (il,0) folds
the term4 contribution into the cB coefficients (cC = 2*cB(il,0)).

Inputs are packed host-side into one [128, CIN] f32 array per core (valid
mode slices only: (0,0) passes through host-side); output is one
[128, COUT] f32 array per core, unpacked host-side into the complex64
(5,5,1024,512) result.
"""

import numpy as np

import bass_rust
import concourse.bass as bass
import concourse.tile as tile
from concourse import mybir
from concourse.bass_utils import run_bass_kernel_spmd
from concourse.vector_clock import ScopedClock

L1 = 5
NX = 1024
NV = 512
NCORES = 8
XS = NX // NCORES  # 128, = SBUF partitions

F32 = mybir.dt.float32
F16 = mybir.dt.float16

# ---------------------------------------------------------------------------
# scal table columns
H1, H2, NH2 = 0, 1, 2          # 0.5*b1, 0.5*b2, -0.5*b2


def A_P(m):                    # +m*b0  (m = 1..4)
    return 3 + (m - 1)


def A_N(m):                    # -m*b0
    return 7 + (m - 1)


def Q1(il):                    # 3*cB0*b1, cB0 = -il(il+1)/2
    return 11 + (il - 1)


def Q2(il):                    # cB0*b2
    return 15 + (il - 1)


def R1(il):                    # cB0*b1
    return 19 + (il - 1)


CB_PAIRS = [(2, 1), (3, 1), (3, 2), (4, 1), (4, 2), (4, 3)]


def CB1(il, im):               # cB*b1
    return 23 + 3 * CB_PAIRS.index((il, im))


def CB2(il, im):               # cB*b2
    return 24 + 3 * CB_PAIRS.index((il, im))


def NCB2(il, im):              # -cB*b2
    return 25 + 3 * CB_PAIRS.index((il, im))


NSCAL = 48  # 41 used, padded

# packed input layout: [scal (NSCAL) | row blocks il=1..4]
# row block: Fr slots (il+1), Fi slots (il+1), D0r, D0i  -- each slot NV cols
IN_OFF = {}
_o = NSCAL
for _il in range(1, L1):
    IN_OFF[_il] = _o
    _o += (2 * (_il + 1) + 2) * NV
CIN = _o

# packed output layout: row blocks il=1..4, each: Dr slots (il+1), Di slots
OUT_OFF = {}
_o = 0
for _il in range(1, L1):
    OUT_OFF[_il] = _o
    _o += 2 * (_il + 1) * NV
COUT = _o


def _cB(il, im):
    return -(il - im) * (il + im + 1) / 2.0


def build_scal(b_sh):
    """b_sh: [XS, 3] float32 -> [XS, NSCAL] float32 coefficient table."""
    b0, b1, b2 = b_sh[:, 0], b_sh[:, 1], b_sh[:, 2]
    s = np.zeros((XS, NSCAL), np.float32)
    s[:, H1] = 0.5 * b1
    s[:, H2] = 0.5 * b2
    s[:, NH2] = -0.5 * b2
    for m in range(1, L1):
        s[:, A_P(m)] = m * b0
        s[:, A_N(m)] = -m * b0
    for il in range(1, L1):
        cB0 = _cB(il, 0)
        s[:, Q1(il)] = 3.0 * cB0 * b1
        s[:, Q2(il)] = cB0 * b2
        s[:, R1(il)] = cB0 * b1
    for (il, im) in CB_PAIRS:
        cB = _cB(il, im)
        s[:, CB1(il, im)] = cB * b1
        s[:, CB2(il, im)] = cB * b2
        s[:, NCB2(il, im)] = -cB * b2
    return s


# ---------------------------------------------------------------------------
# The walrus build in this container rejects instructions carrying more than
# ONE sync-wait ("Too many sync wait commands", setupSyncWait in
# CoreV2/V3GenImpl). Tile's scheduler routinely attaches several. Post-pass:
# hoist all but the last wait of each instruction onto same-engine NOPs
# inserted immediately before it (same basic block, so per-engine program
# order is preserved).
def split_multiwaits(nc):
    for f in nc.m.functions:
        for blk in f.blocks:
            new = []
            changed = False
            for ins in blk.instructions:
                si = ins.sync_info
                if si is not None and len(si.on_wait) > 1:
                    waits = list(si.on_wait)
                    for w in waits[:-1]:
                        nop = mybir.InstNoOp(
                            name=nc.get_next_instruction_name(),
                            engine=ins.engine,
                            bass_nofuse=True,
                            sync_info=mybir.SyncInfo(on_wait=[w],
                                                     on_update=[]),
                        )
                        new.append(nop)
                    ins.sync_info = bass_rust.SyncInfo(
                        on_wait=[waits[-1]], on_update=list(si.on_update))
                    changed = True
                new.append(ins)
            if changed:
                blk.instructions = new


# ---------------------------------------------------------------------------
def _pair(ap, step_elems, nblocks=2):
    """Turn a contiguous [P, L] AP into [P, nblocks, L] with the given
    element step between blocks (may be negative)."""
    c = ap.copy()
    v = c.ap
    last = v.pop()
    v.append((step_elems, nblocks))
    v.append(tuple(last))
    c.ap = v
    return c


def build_bass(split=True):
    """Pair-merged elementwise kernel.

    Per il-row SBUF layout: in-tile [nfi | fr | fi] (ns slots each,
    ns = il+1), out-tile [dr | di]. nfi = -fi (one ACT negate per row) lets
    every b2-coefficient op run as ONE fused scalar_tensor_tensor covering
    BOTH the Dr and Di halves (2-block strided APs, same per-x scalar
    column), halving DVE instruction count. ACT produces the set-term heads
    (b1 pair) and the negates; DVE runs all fused accumulates. Input DMAs
    are chained so row 1 lands first and compute ramps early.
    """
    from bass_rust import add_dep_helper

    MULT = mybir.AluOpType.mult
    ADD = mybir.AluOpType.add

    nc = bass.Bass()
    pin = nc.dram_tensor("pin", [XS, CIN], F16, kind="ExternalInput").ap()
    pscal = nc.dram_tensor("pscal", [XS, NSCAL], F32,
                           kind="ExternalInput").ap()
    pout = nc.dram_tensor("pout", [XS, COUT], F16, kind="ExternalOutput").ap()

    with tile.TileContext(nc) as tc:
        with tc.tile_pool(name="m", bufs=1) as pool:
            scal = pool.tile([XS, NSCAL], F32, tag="scal")
            prev_dma = nc.sync.dma_start(scal[:], pscal[:, 0:NSCAL])

            def sc(col):
                return scal[:, col:col + 1]

            def chain(d):
                nonlocal prev_dma
                add_dep_helper(d.ins, prev_dma.ins,
                               reason="serialize input DMAs")
                prev_dma = d

            row_in = {}
            row_d0 = {}
            for il in range(1, L1):
                ns = il + 1
                t = pool.tile([XS, 3 * ns * NV], F16, tag=f"in{il}")
                d0 = pool.tile([XS, 2 * NV], F16, tag=f"d0_{il}")
                # pin row block: fr slots, fi slots, d0r, d0i
                o = IN_OFF[il]
                chain(nc.sync.dma_start(t[:, ns * NV:3 * ns * NV],
                                        pin[:, o:o + 2 * ns * NV]))
                chain(nc.sync.dma_start(
                    d0[:], pin[:, o + 2 * ns * NV:o + (2 * ns + 2) * NV]))
                row_in[il] = t
                row_d0[il] = d0

            for il in range(1, L1):
                t = row_in[il]
                d0 = row_d0[il]
                ns = il + 1
                nfi = t[:, 0:ns * NV]
                fr = t[:, ns * NV:2 * ns * NV]
                fi = t[:, 2 * ns * NV:3 * ns * NV]
                d0r = d0[:, 0:NV]
                d0i = d0[:, NV:2 * NV]

                to = pool.tile([XS, 2 * ns * NV], F16, tag=f"out{il}")
                dr = to[:, 0:ns * NV]
                di = to[:, ns * NV:2 * ns * NV]

                S = ns * NV  # slot-block stride (elements)

                def sl(buf, k, n=1):
                    return buf[:, k * NV:(k + n) * NV]

                # ACT: nfi = -fi
                nc.scalar.mul(nfi, fi, -1.0)
                # ACT set-b1 head pair:
                #   dr[1..il] = 0.5*b1*fr[0..il-1]; di[1..il] = 0.5*b1*fi[..]
                nc.scalar.mul(_pair(sl(dr, 1, il), S), _pair(sl(fr, 0, il), S),
                              sc(H1))
                # DVE set-b2 pair: dr += 0.5*b2*nfi[0..il-1];
                #                  di += 0.5*b2*fr[0..il-1]
                nc.vector.scalar_tensor_tensor(
                    _pair(sl(dr, 1, il), S), _pair(sl(nfi, 0, il), S),
                    sc(H2), _pair(sl(dr, 1, il), S), MULT, ADD)
                # im=0 bases (distinct b1 scalars -> two singles)
                nc.vector.scalar_tensor_tensor(
                    sl(dr, 0), sl(fr, 1), sc(Q1(il)), d0r, MULT, ADD)
                nc.vector.scalar_tensor_tensor(
                    sl(di, 0), sl(fi, 1), sc(R1(il)), d0i, MULT, ADD)
                # im=0 b2 terms (no negative-stride pairing on HW)
                nc.vector.scalar_tensor_tensor(
                    sl(dr, 0), sl(fi, 1), sc(Q2(il)), sl(dr, 0), MULT, ADD)
                nc.vector.scalar_tensor_tensor(
                    sl(di, 0), sl(fr, 1), sc(Q2(il)), sl(di, 0), MULT, ADD)
                # cB pairs (im=1..il-1)
                for im in range(1, il):
                    nc.vector.scalar_tensor_tensor(
                        _pair(sl(dr, im), S), _pair(sl(fr, im + 1), S),
                        sc(CB1(il, im)), _pair(sl(dr, im), S), MULT, ADD)
                    nc.vector.scalar_tensor_tensor(
                        _pair(sl(dr, im), S), _pair(sl(nfi, im + 1), S),
                        sc(CB2(il, im)), _pair(sl(dr, im), S), MULT, ADD)
                # cA singles
                for im in range(1, il + 1):
                    nc.vector.scalar_tensor_tensor(
                        sl(dr, im), sl(fi, im), sc(A_P(im)), sl(dr, im),
                        MULT, ADD)
                    nc.vector.scalar_tensor_tensor(
                        sl(di, im), sl(fr, im), sc(A_N(im)), sl(di, im),
                        MULT, ADD)

                nc.sync.dma_start(
                    pout[:, OUT_OFF[il]:OUT_OFF[il] + ns * NV], dr)
                nc.sync.dma_start(
                    pout[:, OUT_OFF[il] + ns * NV:OUT_OFF[il] + 2 * ns * NV],
                    di)

    if split:
        split_multiwaits(nc)
    return nc


# ---------------------------------------------------------------------------
def pack_inputs(prev_f_re, prev_f_im, delta0_re, delta0_im, b):
    """-> list of per-core {'pin': [XS, CIN] f32}."""
    in_maps = []
    for c in range(NCORES):
        X = slice(c * XS, (c + 1) * XS)
        p = np.empty((XS, CIN), np.float16)
        p[:, :NSCAL] = 0.0
        pscal = build_scal(np.asarray(b[X], np.float32))
        for il in range(1, L1):
            o = IN_OFF[il]
            ns = il + 1
            p[:, o:o + ns * NV] = (
                np.asarray(prev_f_re[il, :ns, X, :], np.float16)
                .transpose(1, 0, 2).reshape(XS, ns * NV))
            o += ns * NV
            p[:, o:o + ns * NV] = (
                np.asarray(prev_f_im[il, :ns, X, :], np.float16)
                .transpose(1, 0, 2).reshape(XS, ns * NV))
            o += ns * NV
            p[:, o:o + NV] = np.asarray(delta0_re[il, 0, X, :], np.float16)
            o += NV
            p[:, o:o + NV] = np.asarray(delta0_im[il, 0, X, :], np.float16)
        in_maps.append({"pin": p, "pscal": pscal})
    return in_maps


def unpack_outputs(results, delta0_re, delta0_im):
    out = np.zeros((L1, L1, NX, NV), np.complex64)
    out[0, 0] = np.asarray(delta0_re[0, 0]) + 1j * np.asarray(delta0_im[0, 0])
    for c in range(NCORES):
        X = slice(c * XS, (c + 1) * XS)
        p = results[c]["pout"]
        for il in range(1, L1):
            o = OUT_OFF[il]
            ns = il + 1
            dr = p[:, o:o + ns * NV].reshape(XS, ns, NV).transpose(1, 0, 2)
            di = (p[:, o + ns * NV:o + 2 * ns * NV]
                  .reshape(XS, ns, NV).transpose(1, 0, 2))
            out[il, :ns, X, :] = dr.astype(np.float32) + 1j * di.astype(np.float32)
    return out


_NC_CACHE = None


def get_nc():
    global _NC_CACHE
    if _NC_CACHE is None:
        _NC_CACHE = build_bass()
    return _NC_CACHE


def kernel(prev_f_re, prev_f_im, delta0_re, delta0_im, b, v):
    in_maps = pack_inputs(prev_f_re, prev_f_im, delta0_re, delta0_im, b)
    res = run_bass_kernel_spmd(get_nc(), in_maps, list(range(NCORES)))
    return unpack_outputs(res.results, delta0_re, delta0_im)



# revision 7
# speedup vs baseline: 1.5822x; 1.2298x over previous
"""Trainium2 Bass kernel for nn_Bdfdv_51170240364850 (gnn_message_passing).

Computes, for mode pairs (il, im) with im <= il (L1 = 5 modes each way) and
spatial/velocity grid (nx=1024, nv=512):

  D[il,im] = base + (-1j)*im*bx*F[il,im] + cB*bm*F[il,im+1]
             + [im==0] Re(cC*bp*F[il,1])
  base     = 0.5*bm*F[il,im-1]  (il>=1, 1<=im<=il)   else  D0[il,im]

with bx = b[:,0], bm = b[:,1]+1j b[:,2], bp = conj(bm),
cB = -(il-im)(il+im+1)/2, cC = -il(il+1).

Strategy: pure data-parallel over nx across 8 NeuronCores (nx=128 per core on
the 128 SBUF partitions), fp16 I/O, and a three-engine split:

* PE (TensorEngine): every per-x product c(x)*T for modes im>=1 runs as a
  diagonal-weight matmul accumulating in PSUM (diag(c) @ tile scales each
  partition row by c(x)).  Factoring the mode-constant part of each
  coefficient into DVE-prescaled operand tiles (SF = 2cB*F[im+1],
  AF = 2im*F[im]) leaves only FIVE distinct diagonals:
  0.5b1, +-0.5b2, +-0.5b0 - sent prebuilt from the host (tiny).
* DVE: fp16 4x tensor_scalar prescales (SF/AF/G) + the im=0 output rows as
  scalar_tensor_tensor chains (base D0 folded in as the fused add operand).
* ACT: evacuates each finished PSUM bank to the fp16 output tile.

Output slot == one PSUM bank (512 fp32); 20 im>=1 slots stream through the
8 banks via a rotating tile pool.  DMA (fp16, ~37KB in + 28KB out per
partition) is the roofline; all engines run under it.
"""

import numpy as np

import bass_rust
import concourse.bass as bass
import concourse.tile as tile
from concourse import mybir
from concourse.bass_utils import run_bass_kernel_spmd

L1 = 5
NX = 1024
NV = 512
NCORES = 8
XS = NX // NCORES  # 128, = SBUF partitions

F32 = mybir.dt.float32
F16 = mybir.dt.float16

# ---------------------------------------------------------------------------
# slot bookkeeping (im-major ordering of the 14 valid (im, il>=1) F/D slots)
S = [(im, il) for im in range(L1) for il in range(max(1, im), L1)]
SIDX = {s: k for k, s in enumerate(S)}
NS = len(S)                      # 14
S1 = S[4:]                       # the 10 im>=1 slots
RUN = {0: 0, 1: 4, 2: 8, 3: 11, 4: 13}   # start slot index of each im-run
RL = {0: 4, 1: 4, 2: 3, 3: 2, 4: 1}      # run lengths

CB_PAIRS = [(2, 1), (3, 1), (3, 2), (4, 1), (4, 2), (4, 3)]  # (il, im)


def _cB(il, im):
    return -(il - im) * (il + im + 1) / 2.0


# pin layout (fp16): [Fr slots (14 NV) | Fi slots (14) | D0r (4) | D0i (4) |
#                     W diag blocks (5 x 128)]
WOFF = 36 * NV
NDIAG = 5
CIN = WOFF + NDIAG * 128
# pscal (fp32): per-x scalars for the DVE im=0 chain
T1, H1, H2 = 0, 1, 2             # 1.5*b1, 0.5*b1, 0.5*b2
NSCAL = 4
# pout layout (fp16): [Dr slots (14 NV) | Di slots (14 NV)]
COUT = 2 * NS * NV


# ---------------------------------------------------------------------------
# The walrus build in this container rejects instructions carrying more than
# ONE sync-wait ("Too many sync wait commands", setupSyncWait in
# CoreV2/V3GenImpl). Tile's scheduler routinely attaches several. Post-pass:
# hoist all but the last wait of each instruction onto same-engine NOPs
# inserted immediately before it (same basic block, so per-engine program
# order is preserved).
def split_multiwaits(nc):
    for f in nc.m.functions:
        for blk in f.blocks:
            new = []
            changed = False
            for ins in blk.instructions:
                si = ins.sync_info
                if si is not None and len(si.on_wait) > 1:
                    waits = list(si.on_wait)
                    for w in waits[:-1]:
                        nop = mybir.InstNoOp(
                            name=nc.get_next_instruction_name(),
                            engine=ins.engine,
                            bass_nofuse=True,
                            sync_info=mybir.SyncInfo(on_wait=[w],
                                                     on_update=[]),
                        )
                        new.append(nop)
                    ins.sync_info = bass_rust.SyncInfo(
                        on_wait=[waits[-1]], on_update=list(si.on_update))
                    changed = True
                new.append(ins)
            if changed:
                blk.instructions = new


# ---------------------------------------------------------------------------
def _pair(ap, step_elems, nblocks=2):
    """Turn a contiguous [P, L] AP into [P, nblocks, L] with the given
    element step between blocks."""
    c = ap.copy()
    v = c.ap
    last = v.pop()
    v.append((step_elems, nblocks))
    v.append(tuple(last))
    c.ap = v
    return c


def build_bass(split=True):
    from bass_rust import add_dep_helper

    MULT = mybir.AluOpType.mult
    ADD = mybir.AluOpType.add

    nc = bass.Bass()
    pin = nc.dram_tensor("pin", [XS, CIN], F16, kind="ExternalInput").ap()
    pscal = nc.dram_tensor("pscal", [XS, NSCAL], F32,
                           kind="ExternalInput").ap()
    pout = nc.dram_tensor("pout", [XS, COUT], F16, kind="ExternalOutput").ap()

    with tile.TileContext(nc) as tc:
        with tc.tile_pool(name="m", bufs=1) as pool, \
             tc.psum_pool(name="p", bufs=8) as ppool:
            fF = pool.tile([XS, 2 * NS * NV], F16, tag="fF")
            fD0 = pool.tile([XS, 8 * NV], F16, tag="fD0")
            fW = pool.tile([XS, NDIAG * 128], F16, tag="fW")
            scal = pool.tile([XS, NSCAL], F32, tag="scal")
            AF = pool.tile([XS, 2 * 10 * NV], F16, tag="AF")
            SF = pool.tile([XS, 2 * 6 * NV], F16, tag="SF")
            G = pool.tile([XS, 2 * 4 * NV], F16, tag="G")
            OUT = pool.tile([XS, 2 * NS * NV], F16, tag="OUT")

            def fr(k):
                return fF[:, k * NV:(k + 1) * NV]

            def fi(k):
                return fF[:, (NS + k) * NV:(NS + k + 1) * NV]

            def sfr(j):
                return SF[:, j * NV:(j + 1) * NV]

            def sfi(j):
                return SF[:, (6 + j) * NV:(7 + j) * NV]

            def afr(a):
                return AF[:, a * NV:(a + 1) * NV]

            def afi(a):
                return AF[:, (10 + a) * NV:(11 + a) * NV]

            def W(j):
                return fW[:, j * 128:(j + 1) * 128]

            def outr(k, n=1):
                return OUT[:, k * NV:(k + n) * NV]

            def outi(k, n=1):
                return OUT[:, (NS + k) * NV:(NS + k + n) * NV]

            def sc(col):
                return scal[:, col:col + 1]

            # ---- input DMAs, chained so early groups land first ----
            prev_dma = nc.sync.dma_start(scal[:], pscal[:])

            def chain(d):
                nonlocal prev_dma
                add_dep_helper(d.ins, prev_dma.ins,
                               reason="serialize input DMAs")
                prev_dma = d

            chain(nc.sync.dma_start(fW[:], pin[:, WOFF:WOFF + NDIAG * 128]))

            def in_run(m, imag):
                o = (NS * imag + RUN[m]) * NV
                n = RL[m] * NV
                chain(nc.sync.dma_start(fF[:, o:o + n], pin[:, o:o + n]))

            in_run(0, 0)
            in_run(0, 1)
            in_run(1, 0)
            in_run(1, 1)
            in_run(2, 0)
            in_run(2, 1)
            chain(nc.sync.dma_start(fD0[:],
                                    pin[:, 28 * NV:36 * NV]))
            in_run(3, 0)
            in_run(3, 1)
            in_run(4, 0)
            in_run(4, 1)

            # ---- DVE prescale helpers (fp16 tensor_scalar at 4x) ----
            def presc_AF(m):
                a = RUN[m] - 4
                n = RL[m] * NV
                nc.vector.tensor_scalar_mul(
                    _pair(AF[:, a * NV:a * NV + n], 10 * NV),
                    _pair(fF[:, RUN[m] * NV:RUN[m] * NV + n], NS * NV),
                    2.0 * m)

            def presc_SF(j):
                il, im = CB_PAIRS[j]
                ksrc = SIDX[(im + 1, il)]
                nc.vector.tensor_scalar_mul(
                    _pair(sfr(j), 6 * NV), _pair(fr(ksrc), NS * NV),
                    2.0 * _cB(il, im))

            def presc_G(il):
                k1 = SIDX[(1, il)]
                nc.vector.tensor_scalar_mul(
                    _pair(G[:, (il - 1) * NV:il * NV], 4 * NV),
                    _pair(fr(k1), NS * NV),
                    float(-il * (il + 1)))

            SF_OF_GROUP = {1: [0, 1, 3], 2: [2, 4], 3: [5], 4: []}

            # ---- per im-group: prescales, PE matmul chains, ACT evac ----
            for m in range(1, L1):
                presc_AF(m)
                for j in SF_OF_GROUP[m]:
                    presc_SF(j)
                if m == 1:
                    for il in range(1, L1):
                        presc_G(il)

                ils = list(range(m, L1))
                bankR = {}
                bankI = {}
                for il in ils:
                    bankR[il] = ppool.tile([XS, NV], F32, tag="bk",
                                           name=f"bkR{m}_{il}")
                    bankI[il] = ppool.tile([XS, NV], F32, tag="bk",
                                           name=f"bkI{m}_{il}")

                def mm(bank, j, rhs, start=False, stop=False):
                    nc.tensor.matmul(bank[:], W(j), rhs, start=start,
                                     stop=stop, skip_group_check=True)

                # D1 = 0.5*b1 pass (first write of every bank)
                for il in ils:
                    kp = SIDX[(m - 1, il)]
                    mm(bankR[il], 0, fr(kp), start=True)
                    mm(bankI[il], 0, fi(kp), start=True)
                for il in ils:
                    if il > m:
                        j = CB_PAIRS.index((il, m))
                        mm(bankR[il], 0, sfr(j))
                        mm(bankI[il], 0, sfi(j))
                # D2 = +0.5*b2 (imag accumulators)
                for il in ils:
                    mm(bankI[il], 1, fr(SIDX[(m - 1, il)]))
                    if il > m:
                        mm(bankI[il], 1, sfr(CB_PAIRS.index((il, m))))
                # D3 = -0.5*b2 (real accumulators)
                for il in ils:
                    mm(bankR[il], 2, fi(SIDX[(m - 1, il)]))
                    if il > m:
                        mm(bankR[il], 2, sfi(CB_PAIRS.index((il, m))))
                # D4 = +0.5*b0 closes real banks; D5 = -0.5*b0 closes imag
                for il in ils:
                    a = RUN[m] - 4 + (il - m)
                    mm(bankR[il], 3, afi(a), stop=True)
                for il in ils:
                    a = RUN[m] - 4 + (il - m)
                    mm(bankI[il], 4, afr(a), stop=True)

                # ACT evacuation (fp32 PSUM -> fp16 OUT), then output DMA
                for il in ils:
                    nc.scalar.copy(outr(SIDX[(m, il)]), bankR[il][:])
                for il in ils:
                    nc.scalar.copy(outi(SIDX[(m, il)]), bankI[il][:])
                o = RUN[m] * NV
                n = RL[m] * NV
                nc.sync.dma_start(pout[:, o:o + n], OUT[:, o:o + n])
                nc.sync.dma_start(pout[:, NS * NV + o:NS * NV + o + n],
                                  OUT[:, NS * NV + o:NS * NV + o + n])

            # ---- im = 0 rows on DVE: D[0,il] = D0 + bm*(cB0-weighted F1) ----
            gr = G[:, 0:4 * NV]
            gi = G[:, 4 * NV:8 * NV]
            d0r = fD0[:, 0:4 * NV]
            d0i = fD0[:, 4 * NV:8 * NV]
            nc.vector.scalar_tensor_tensor(
                outr(0, 4), gr, sc(T1), d0r, MULT, ADD)
            nc.vector.scalar_tensor_tensor(
                outr(0, 4), gi, sc(H2), outr(0, 4), MULT, ADD)
            nc.vector.scalar_tensor_tensor(
                outi(0, 4), gi, sc(H1), d0i, MULT, ADD)
            nc.vector.scalar_tensor_tensor(
                outi(0, 4), gr, sc(H2), outi(0, 4), MULT, ADD)
            nc.sync.dma_start(pout[:, 0:4 * NV], outr(0, 4))
            nc.sync.dma_start(pout[:, NS * NV:(NS + 4) * NV], outi(0, 4))

    if split:
        split_multiwaits(nc)
    return nc


# ---------------------------------------------------------------------------
def pack_inputs(prev_f_re, prev_f_im, delta0_re, delta0_im, b):
    """-> list of per-core {'pin': [XS, CIN] f16, 'pscal': [XS, 4] f32}."""
    pr = np.asarray(prev_f_re, np.float32)
    pi = np.asarray(prev_f_im, np.float32)
    d0r = np.asarray(delta0_re, np.float32)
    d0i = np.asarray(delta0_im, np.float32)
    bb = np.asarray(b, np.float32)
    ar = np.arange(XS)
    in_maps = []
    for c in range(NCORES):
        X = slice(c * XS, (c + 1) * XS)
        p = np.zeros((XS, CIN), np.float16)
        for k, (im, il) in enumerate(S):
            p[:, k * NV:(k + 1) * NV] = pr[il, im, X, :]
            p[:, (NS + k) * NV:(NS + k + 1) * NV] = pi[il, im, X, :]
        for il in range(1, L1):
            p[:, (28 + il - 1) * NV:(29 + il - 1) * NV] = d0r[il, 0, X, :]
            p[:, (32 + il - 1) * NV:(33 + il - 1) * NV] = d0i[il, 0, X, :]
        b0, b1, b2 = bb[X, 0], bb[X, 1], bb[X, 2]
        for j, cx in enumerate([0.5 * b1, 0.5 * b2, -0.5 * b2,
                                0.5 * b0, -0.5 * b0]):
            blk = np.zeros((XS, 128), np.float16)
            blk[ar, ar] = cx.astype(np.float16)
            p[:, WOFF + j * 128:WOFF + (j + 1) * 128] = blk
        ps = np.zeros((XS, NSCAL), np.float32)
        ps[:, T1] = 1.5 * b1
        ps[:, H1] = 0.5 * b1
        ps[:, H2] = 0.5 * b2
        in_maps.append({"pin": p, "pscal": ps})
    return in_maps


def unpack_outputs(results, delta0_re, delta0_im):
    out = np.zeros((L1, L1, NX, NV), np.complex64)
    out[0, 0] = np.asarray(delta0_re[0, 0]) + 1j * np.asarray(delta0_im[0, 0])
    for c in range(NCORES):
        X = slice(c * XS, (c + 1) * XS)
        p = results[c]["pout"]
        for k, (im, il) in enumerate(S):
            dr = p[:, k * NV:(k + 1) * NV].astype(np.float32)
            di = p[:, (NS + k) * NV:(NS + k + 1) * NV].astype(np.float32)
            out[il, im, X, :] = dr + 1j * di
    return out


_NC_CACHE = None


def get_nc():
    global _NC_CACHE
    if _NC_CACHE is None:
        _NC_CACHE = build_bass()
    return _NC_CACHE


def kernel(prev_f_re, prev_f_im, delta0_re, delta0_im, b, v):
    in_maps = pack_inputs(prev_f_re, prev_f_im, delta0_re, delta0_im, b)
    res = run_bass_kernel_spmd(get_nc(), in_maps, list(range(NCORES)))
    return unpack_outputs(res.results, delta0_re, delta0_im)


# revision 8
# speedup vs baseline: 2.1263x; 1.3439x over previous
"""Trainium2 Bass kernel for nn_Bdfdv_51170240364850 (gnn_message_passing).

Computes, for mode pairs (il, im) with im <= il (L1 = 5 modes each way) and
spatial/velocity grid (nx=1024, nv=512):

  D[il,im] = base + (-1j)*im*bx*F[il,im] + cB*bm*F[il,im+1]
             + [im==0] Re(cC*bp*F[il,1])
  base     = 0.5*bm*F[il,im-1]  (il>=1, 1<=im<=il)   else  D0[il,im]

with bx = b[:,0], bm = b[:,1]+1j b[:,2], bp = conj(bm),
cB = -(il-im)(il+im+1)/2, cC = -il(il+1).

Strategy: pure data-parallel over nx across 8 NeuronCores (nx=128 per core on
the 128 SBUF partitions), fp16 I/O, and a three-engine split:

* PE (TensorEngine): every per-x product c(x)*T for modes im>=1 runs as a
  diagonal-weight matmul accumulating in PSUM (diag(c) @ tile scales each
  partition row by c(x)).  Factoring the mode-constant part of each
  coefficient into DVE-prescaled operand tiles (SF = 2cB*F[im+1],
  AF = 2im*F[im]) leaves only FIVE distinct diagonals:
  0.5b1, +-0.5b2, +-0.5b0 - sent prebuilt from the host (tiny).
* DVE: fp16 4x tensor_scalar prescales (SF/AF/G) + the im=0 output rows as
  scalar_tensor_tensor chains (base D0 folded in as the fused add operand).
* ACT: evacuates each finished PSUM bank to the fp16 output tile.

Output slot == one PSUM bank (512 fp32); 20 im>=1 slots stream through the
8 banks via a rotating tile pool.  DMA (fp16, ~37KB in + 28KB out per
partition) is the roofline; all engines run under it.
"""

import numpy as np

import bass_rust
import concourse.bass as bass
import concourse.tile as tile
from concourse import mybir
from concourse.bass_utils import run_bass_kernel_spmd

L1 = 5
NX = 1024
NV = 512
NCORES = 8
XS = NX // NCORES  # 128, = SBUF partitions

F32 = mybir.dt.float32
F16 = mybir.dt.float16

# ---------------------------------------------------------------------------
# slot bookkeeping (im-major ordering of the 14 valid (im, il>=1) F/D slots)
S = [(im, il) for im in range(L1) for il in range(max(1, im), L1)]
SIDX = {s: k for k, s in enumerate(S)}
NS = len(S)                      # 14
S1 = S[4:]                       # the 10 im>=1 slots
RUN = {0: 0, 1: 4, 2: 8, 3: 11, 4: 13}   # start slot index of each im-run
RL = {0: 4, 1: 4, 2: 3, 3: 2, 4: 1}      # run lengths

CB_PAIRS = [(2, 1), (3, 1), (3, 2), (4, 1), (4, 2), (4, 3)]  # (il, im)

# F/D run-interleaved layout: run m holds [re slots | im slots] back-to-back,
# so each im-run moves as ONE contiguous DMA.
FOFF = {}
_o = 0
for _m in range(L1):
    FOFF[_m] = _o
    _o += 2 * RL[_m] * NV
assert _o == 2 * NS * NV


def _cB(il, im):
    return -(il - im) * (il + im + 1) / 2.0


# pin layout (fp16): [Fr slots (14 NV) | Fi slots (14) | D0r (4) | D0i (4) |
#                     W diag blocks (5 x 128)]
WOFF = 36 * NV
NDIAG = 5
CIN = WOFF + NDIAG * 128
# pscal (fp32): per-x scalars for the DVE im=0 chain
T1, H1, H2 = 0, 1, 2             # 1.5*b1, 0.5*b1, 0.5*b2
NSCAL = 4
# pout layout (fp16): [Dr slots (14 NV) | Di slots (14 NV)]
COUT = 2 * NS * NV


# ---------------------------------------------------------------------------
# The walrus build in this container rejects instructions carrying more than
# ONE sync-wait ("Too many sync wait commands", setupSyncWait in
# CoreV2/V3GenImpl). Tile's scheduler routinely attaches several. Post-pass:
# hoist all but the last wait of each instruction onto same-engine NOPs
# inserted immediately before it (same basic block, so per-engine program
# order is preserved).
def split_multiwaits(nc):
    for f in nc.m.functions:
        for blk in f.blocks:
            new = []
            changed = False
            for ins in blk.instructions:
                si = ins.sync_info
                if si is not None and len(si.on_wait) > 1:
                    waits = list(si.on_wait)
                    for w in waits[:-1]:
                        nop = mybir.InstNoOp(
                            name=nc.get_next_instruction_name(),
                            engine=ins.engine,
                            bass_nofuse=True,
                            sync_info=mybir.SyncInfo(on_wait=[w],
                                                     on_update=[]),
                        )
                        new.append(nop)
                    ins.sync_info = bass_rust.SyncInfo(
                        on_wait=[waits[-1]], on_update=list(si.on_update))
                    changed = True
                new.append(ins)
            if changed:
                blk.instructions = new


# ---------------------------------------------------------------------------
def _pair(ap, step_elems, nblocks=2):
    """Turn a contiguous [P, L] AP into [P, nblocks, L] with the given
    element step between blocks."""
    c = ap.copy()
    v = c.ap
    last = v.pop()
    v.append((step_elems, nblocks))
    v.append(tuple(last))
    c.ap = v
    return c


def build_bass(split=True):
    MULT = mybir.AluOpType.mult
    ADD = mybir.AluOpType.add

    nc = bass.Bass()
    pin = nc.dram_tensor("pin", [XS, CIN], F16, kind="ExternalInput").ap()
    pscal = nc.dram_tensor("pscal", [XS, NSCAL], F32,
                           kind="ExternalInput").ap()
    pout = nc.dram_tensor("pout", [XS, COUT], F16, kind="ExternalOutput").ap()

    with tile.TileContext(nc) as tc:
        with tc.tile_pool(name="m", bufs=1) as pool, \
             tc.psum_pool(name="p", bufs=8) as ppool:
            fF = pool.tile([XS, 2 * NS * NV], F16, tag="fF")
            fD0 = pool.tile([XS, 8 * NV], F16, tag="fD0")
            fW = pool.tile([XS, NDIAG * 128], F16, tag="fW")
            scal = pool.tile([XS, NSCAL], F32, tag="scal")
            AF = pool.tile([XS, 2 * 10 * NV], F16, tag="AF")
            SF = pool.tile([XS, 2 * 6 * NV], F16, tag="SF")
            G = pool.tile([XS, 2 * 4 * NV], F16, tag="G")
            OUT = pool.tile([XS, 2 * NS * NV], F16, tag="OUT")

            def fslot(k, imag, n=1):
                m = S[k][0]
                o = FOFF[m] + (imag * RL[m] + (k - RUN[m])) * NV
                return fF[:, o:o + n * NV]

            def fr(k):
                return fslot(k, 0)

            def fi(k):
                return fslot(k, 1)

            def sfr(j):
                return SF[:, j * NV:(j + 1) * NV]

            def sfi(j):
                return SF[:, (6 + j) * NV:(7 + j) * NV]

            def afr(a):
                return AF[:, a * NV:(a + 1) * NV]

            def afi(a):
                return AF[:, (10 + a) * NV:(11 + a) * NV]

            def W(j):
                return fW[:, j * 128:(j + 1) * 128]

            def outr(k, n=1):
                m = S[k][0]
                o = FOFF[m] + (k - RUN[m]) * NV
                return OUT[:, o:o + n * NV]

            def outi(k, n=1):
                m = S[k][0]
                o = FOFF[m] + (RL[m] + k - RUN[m]) * NV
                return OUT[:, o:o + n * NV]

            def sc(col):
                return scal[:, col:col + 1]

            # ---- input DMAs: issued back-to-back (no completion chain);
            # the queues drain descriptors in issue order, so emission order
            # IS the priority order without per-link round-trip latency.
            nc.sync.dma_start(scal[:], pscal[:])
            nc.sync.dma_start(fW[:], pin[:, WOFF:WOFF + NDIAG * 128])

            def in_run(m):
                o = FOFF[m]
                n = 2 * RL[m] * NV
                nc.sync.dma_start(fF[:, o:o + n], pin[:, o:o + n])

            in_run(0)
            in_run(1)
            in_run(2)
            nc.sync.dma_start(fD0[:], pin[:, 28 * NV:36 * NV])
            in_run(3)
            in_run(4)

            # ---- DVE prescale helpers (fp16 tensor_scalar at 4x) ----
            def presc_AF(m):
                a = RUN[m] - 4
                n = RL[m] * NV
                nc.vector.tensor_scalar_mul(
                    _pair(AF[:, a * NV:a * NV + n], 10 * NV),
                    _pair(fslot(RUN[m], 0, RL[m]), RL[m] * NV),
                    2.0 * m)

            def presc_SF(j):
                il, im = CB_PAIRS[j]
                ksrc = SIDX[(im + 1, il)]
                nc.vector.tensor_scalar_mul(
                    _pair(sfr(j), 6 * NV),
                    _pair(fr(ksrc), RL[im + 1] * NV),
                    2.0 * _cB(il, im))

            def presc_G(il):
                k1 = SIDX[(1, il)]
                nc.vector.tensor_scalar_mul(
                    _pair(G[:, (il - 1) * NV:il * NV], 4 * NV),
                    _pair(fr(k1), RL[1] * NV),
                    float(-il * (il + 1)))

            SF_OF_GROUP = {1: [0, 1, 3], 2: [2, 4], 3: [5], 4: []}

            # ---- per im-group: prescales, PE matmul chains, ACT evac ----
            for m in range(1, L1):
                presc_AF(m)
                for j in SF_OF_GROUP[m]:
                    presc_SF(j)
                if m == 1:
                    for il in range(1, L1):
                        presc_G(il)

                ils = list(range(m, L1))
                bankR = {}
                bankI = {}
                for il in ils:
                    bankR[il] = ppool.tile([XS, NV], F32, tag="bk",
                                           name=f"bkR{m}_{il}")
                    bankI[il] = ppool.tile([XS, NV], F32, tag="bk",
                                           name=f"bkI{m}_{il}")

                def mm(bank, j, rhs, start=False, stop=False):
                    nc.tensor.matmul(bank[:], W(j), rhs, start=start,
                                     stop=stop, skip_group_check=True)

                # D1 = 0.5*b1 pass (first write of every bank)
                for il in ils:
                    kp = SIDX[(m - 1, il)]
                    mm(bankR[il], 0, fr(kp), start=True)
                    mm(bankI[il], 0, fi(kp), start=True)
                for il in ils:
                    if il > m:
                        j = CB_PAIRS.index((il, m))
                        mm(bankR[il], 0, sfr(j))
                        mm(bankI[il], 0, sfi(j))
                # D2 = +0.5*b2 (imag accumulators)
                for il in ils:
                    mm(bankI[il], 1, fr(SIDX[(m - 1, il)]))
                    if il > m:
                        mm(bankI[il], 1, sfr(CB_PAIRS.index((il, m))))
                # D3 = -0.5*b2 (real accumulators)
                for il in ils:
                    mm(bankR[il], 2, fi(SIDX[(m - 1, il)]))
                    if il > m:
                        mm(bankR[il], 2, sfi(CB_PAIRS.index((il, m))))
                # D4 = +0.5*b0 closes real banks; D5 = -0.5*b0 closes imag
                for il in ils:
                    a = RUN[m] - 4 + (il - m)
                    mm(bankR[il], 3, afi(a), stop=True)
                for il in ils:
                    a = RUN[m] - 4 + (il - m)
                    mm(bankI[il], 4, afr(a), stop=True)

                # ACT evacuation (fp32 PSUM -> fp16 OUT), then output DMA
                for il in ils:
                    nc.scalar.copy(outr(SIDX[(m, il)]), bankR[il][:])
                for il in ils:
                    nc.scalar.copy(outi(SIDX[(m, il)]), bankI[il][:])
                o = FOFF[m]
                n = 2 * RL[m] * NV
                nc.sync.dma_start(pout[:, o:o + n], OUT[:, o:o + n])

            # ---- im = 0 rows on DVE: D[0,il] = D0 + bm*(cB0-weighted F1) ----
            gr = G[:, 0:4 * NV]
            gi = G[:, 4 * NV:8 * NV]
            d0r = fD0[:, 0:4 * NV]
            d0i = fD0[:, 4 * NV:8 * NV]
            nc.vector.scalar_tensor_tensor(
                outr(0, 4), gr, sc(T1), d0r, MULT, ADD)
            nc.vector.scalar_tensor_tensor(
                outr(0, 4), gi, sc(H2), outr(0, 4), MULT, ADD)
            nc.vector.scalar_tensor_tensor(
                outi(0, 4), gi, sc(H1), d0i, MULT, ADD)
            nc.vector.scalar_tensor_tensor(
                outi(0, 4), gr, sc(H2), outi(0, 4), MULT, ADD)
            nc.sync.dma_start(pout[:, 0:8 * NV], OUT[:, 0:8 * NV])

    if split:
        split_multiwaits(nc)
    return nc


# ---------------------------------------------------------------------------
def pack_inputs(prev_f_re, prev_f_im, delta0_re, delta0_im, b):
    """-> list of per-core {'pin': [XS, CIN] f16, 'pscal': [XS, 4] f32}."""
    pr = np.asarray(prev_f_re, np.float32)
    pi = np.asarray(prev_f_im, np.float32)
    d0r = np.asarray(delta0_re, np.float32)
    d0i = np.asarray(delta0_im, np.float32)
    bb = np.asarray(b, np.float32)
    ar = np.arange(XS)
    in_maps = []
    for c in range(NCORES):
        X = slice(c * XS, (c + 1) * XS)
        p = np.zeros((XS, CIN), np.float16)
        for k, (im, il) in enumerate(S):
            o = FOFF[im] + (k - RUN[im]) * NV
            p[:, o:o + NV] = pr[il, im, X, :]
            o += RL[im] * NV
            p[:, o:o + NV] = pi[il, im, X, :]
        for il in range(1, L1):
            p[:, (28 + il - 1) * NV:(29 + il - 1) * NV] = d0r[il, 0, X, :]
            p[:, (32 + il - 1) * NV:(33 + il - 1) * NV] = d0i[il, 0, X, :]
        b0, b1, b2 = bb[X, 0], bb[X, 1], bb[X, 2]
        for j, cx in enumerate([0.5 * b1, 0.5 * b2, -0.5 * b2,
                                0.5 * b0, -0.5 * b0]):
            blk = np.zeros((XS, 128), np.float16)
            blk[ar, ar] = cx.astype(np.float16)
            p[:, WOFF + j * 128:WOFF + (j + 1) * 128] = blk
        ps = np.zeros((XS, NSCAL), np.float32)
        ps[:, T1] = 1.5 * b1
        ps[:, H1] = 0.5 * b1
        ps[:, H2] = 0.5 * b2
        in_maps.append({"pin": p, "pscal": ps})
    return in_maps


def unpack_outputs(results, delta0_re, delta0_im):
    out = np.zeros((L1, L1, NX, NV), np.complex64)
    out[0, 0] = np.asarray(delta0_re[0, 0]) + 1j * np.asarray(delta0_im[0, 0])
    for c in range(NCORES):
        X = slice(c * XS, (c + 1) * XS)
        p = results[c]["pout"]
        for k, (im, il) in enumerate(S):
            o = FOFF[im] + (k - RUN[im]) * NV
            dr = p[:, o:o + NV].astype(np.float32)
            o += RL[im] * NV
            di = p[:, o:o + NV].astype(np.float32)
            out[il, im, X, :] = dr + 1j * di
    return out


_NC_CACHE = None


def get_nc():
    global _NC_CACHE
    if _NC_CACHE is None:
        _NC_CACHE = build_bass()
    return _NC_CACHE


def kernel(prev_f_re, prev_f_im, delta0_re, delta0_im, b, v):
    in_maps = pack_inputs(prev_f_re, prev_f_im, delta0_re, delta0_im, b)
    res = run_bass_kernel_spmd(get_nc(), in_maps, list(range(NCORES)))
    return unpack_outputs(res.results, delta0_re, delta0_im)
